# revision 1
# baseline (speedup 1.0000x reference)
"""Trainium2 Bass kernel: GNN edge decoder (nn_Decoder).

Computation (per edge e):
    emb  = concat(X[src[e]], X[dst[e]])          # [256]
    h    = relu(emb @ W1.T + b1)                 # [128]
    logit= h @ W2.T + b2                         # scalar
Outputs: (logits[E], labels[E]=ones)

Device strategy (8 cores, data-parallel over edges):

Phase 1 (Tile, replicated): precompute node table in fp16
    P[n] = [ X[n] @ W1a.T + b1  ||  X[n] @ W1b.T ]     ([N,256], row=512B)
  via per-tile PE transpose of X + one matmul with [W1a.T||W1b.T], bias folded
  in with a K=1 accumulate matmul.

Phase 2 (raw bass): per-edge gather + MLP tail.
  The only fast gather on TRN2 is the GPSIMD CounterMachine `dma_gather`
  (int16 indices, <=1024 per call), so the HOST bins each core's edges into
  16 groups by (src_range, dst_range) over 4 node ranges of 25024 rows;
  within a group both gathers use bin-local int16 indices. Groups are padded
  to a fixed quota (7 calls x 1024). Per 1024-edge unit:
    dma_gather A <- P[src_bin rows, 0:128], dma_gather B <- P[dst_bin, 128:256]
    DVE: s = A + B;  prod = max(s,0) * w2;  logits = sum(prod) + b2
    ACT: DMA logits out.
  Host un-permutes logits and drops pad slots.
"""

import numpy as np

D = 128
N_NODES = 100000
E_TOTAL = 800000
N_CORES = 8

FULL_CFG = dict(
    n_pad=100096,      # 782*128 node rows (padded)
    chunk_t=34,        # node tiles per precompute chunk
    n_chunks=23,       # 23*34*128 = 100096
    bin_rows=25024,    # node rows per bin (4 bins)
    qg=1024,           # indices per dma_gather call (hw limit)
    calls_per_group=7, # quota = 7*1024 = 7168 slots per group
    depth=14,          # gather units in flight (must divide 112)
    lbuf=8,            # logits tiles in flight
)
N_GROUPS = 16


def _units(cfg):
    return N_GROUPS * cfg["calls_per_group"]


def _slots(cfg):
    return _units(cfg) * cfg["qg"]


def build_bass(cfg=None, n_reps=1, n_reps_p1=None, n_reps_p2=None, tail=True):
    from contextlib import ExitStack

    import concourse.bacc as bacc
    import concourse.tile as tile
    from concourse import bass, library_config, mybir
    from concourse.masks import make_identity

    cfg = cfg or FULL_CFG
    n_pad = cfg["n_pad"]
    chunk_t = cfg["chunk_t"]
    n_chunks = cfg["n_chunks"]
    BINR = cfg["bin_rows"]
    QG = cfg["qg"]
    CPG = cfg["calls_per_group"]
    DEPTH = cfg["depth"]
    LBUF = cfg["lbuf"]
    NU = _units(cfg)
    assert n_chunks * chunk_t * 128 == n_pad
    assert 4 * BINR == n_pad
    assert QG % 128 == 0 and QG <= 1024
    assert NU % DEPTH == 0 and NU % LBUF == 0
    JPU = QG // 128           # logits free cols per unit
    SPU = QG // 16            # idx cols per call

    fp16 = mybir.dt.float16
    f32 = mybir.dt.float32
    i16 = mybir.dt.int16
    ALU = mybir.AluOpType

    nc = bacc.Bacc(
        "TRN2", target_bir_lowering=False, debug=False, num_devices=N_CORES,
        num_swdge_queues=2, dynamic_dma_scratch_size=32768,
    )

    x_d = nc.dram_tensor("x", [n_pad, D], f32, kind="ExternalInput").ap()
    w1_d = nc.dram_tensor("w1", [D, 2 * D], f32, kind="ExternalInput").ap()
    b1_d = nc.dram_tensor("b1", [D], f32, kind="ExternalInput").ap()
    w2_d = nc.dram_tensor("w2", [1, D], f32, kind="ExternalInput").ap()
    b2_d = nc.dram_tensor("b2", [1], f32, kind="ExternalInput").ap()
    gidx_d = nc.dram_tensor("gidx", [128, NU * 2 * SPU], i16, kind="ExternalInput").ap()
    out_d = nc.dram_tensor("logits", [_slots(cfg)], f32, kind="ExternalOutput").ap()
    p_d = nc.dram_tensor("ptab", [n_pad, 2 * D], fp16).ap()

    # persistent SBUF for phase-2 (written during phase 1)
    w2rep_t = nc.alloc_sbuf_tensor("w2rep", [128, D], fp16)
    b2bc_t = nc.alloc_sbuf_tensor("b2bc", [128, 1], f32)
    idx_all_t = nc.alloc_sbuf_tensor("idx_all", [128, NU * 2 * SPU], i16)
    dstA_t = nc.alloc_sbuf_tensor("dstA", [128, DEPTH * JPU, 128], fp16)
    dstB_t = nc.alloc_sbuf_tensor("dstB", [128, DEPTH * JPU, 128], fp16)
    red1_t = nc.alloc_sbuf_tensor("red1", [128, JPU * 16], fp16)
    lg_t = nc.alloc_sbuf_tensor("lg", [128, LBUF * JPU], f32)
    w2rep = w2rep_t.ap()
    b2bc = b2bc_t.ap()
    idx_all = idx_all_t.ap()
    dstA = dstA_t.ap()
    dstB = dstB_t.ap()
    red1 = red1_t.ap()
    lg = lg_t.ap()

    if n_reps_p1 is None:
        n_reps_p1 = n_reps
    if n_reps_p2 is None:
        n_reps_p2 = n_reps

    with ExitStack() as top:
        idx_sem = top.enter_context(nc.semaphore("idx_sem"))
        dv = top.enter_context(nc.semaphore("dv"))
        gsems = [
            top.enter_context(nc.semaphore(f"gs{k}")) for k in range(DEPTH)
        ]
        olsems = [
            top.enter_context(nc.semaphore(f"ol{k}")) for k in range(LBUF)
        ]

        for rep in range(n_reps_p1):
            # ---------------- phase 1 (Tile): precompute P -----------------
            with ExitStack() as ctx:
                tc = ctx.enter_context(tile.TileContext(nc))
                const = ctx.enter_context(tc.tile_pool(name="const", bufs=1))

                ident = const.tile([128, 128], fp16)
                make_identity(nc, ident[:])

                w1f = const.tile([128, 2 * D], f32)
                nc.sync.dma_start(out=w1f[:], in_=w1_d)
                w1h = const.tile([128, 2 * D], fp16)
                nc.vector.tensor_copy(out=w1h[:], in_=w1f[:])
                wcatT = const.tile([128, 2 * D], fp16)

                b1f = const.tile([1, D], f32)
                nc.sync.dma_start(out=b1f[:], in_=b1_d.unsqueeze(0))
                b1cat = const.tile([1, 2 * D], fp16)
                nc.vector.memset(b1cat[:], 0.0)
                nc.vector.tensor_copy(out=b1cat[:, 0:D], in_=b1f[:])

                w2f = const.tile([1, D], f32)
                nc.sync.dma_start(out=w2f[:], in_=w2_d)
                w2h = const.tile([1, D], fp16)
                nc.vector.tensor_copy(out=w2h[:], in_=w2f[:])
                b2f = const.tile([1, 1], f32)
                nc.sync.dma_start(out=b2f[:], in_=b2_d.unsqueeze(0))

                onesh = const.tile([1, D], fp16)
                nc.vector.memset(onesh[:], 1.0)
                ones32 = const.tile([1, 128], f32)
                nc.vector.memset(ones32[:], 1.0)

                with tc.tile_pool(name="ps_setup", bufs=1, space="PSUM") as pss:
                    for half in range(2):
                        tp = pss.tile([128, 128], fp16, tag="t")
                        nc.tensor.transpose(
                            tp[:], w1h[:, half * 128 : (half + 1) * 128], ident[:]
                        )
                        nc.scalar.copy(wcatT[:, half * 128 : (half + 1) * 128], tp[:])

                    wps = pss.tile([128, D], f32, tag="b")
                    nc.tensor.matmul(
                        wps[:], lhsT=onesh[:], rhs=w2h[:], start=True, stop=True
                    )
                    nc.vector.tensor_copy(out=w2rep, in_=wps[:])

                    b2ps = pss.tile([128, 1], f32, tag="s")
                    nc.tensor.matmul(
                        b2ps[:], lhsT=ones32[:], rhs=b2f[:], start=True, stop=True
                    )
                    nc.vector.tensor_copy(out=b2bc, in_=b2ps[:])

                xw_pool = ctx.enter_context(tc.tile_pool(name="xw", bufs=2))
                pw_pool = ctx.enter_context(tc.tile_pool(name="pw", bufs=2))
                xt_pool = ctx.enter_context(tc.tile_pool(name="xt", bufs=3))
                psA = ctx.enter_context(tc.tile_pool(name="psA", bufs=3, space="PSUM"))
                psB = ctx.enter_context(tc.tile_pool(name="psB", bufs=3, space="PSUM"))

                rpc = chunk_t * 128
                for c in range(n_chunks):
                    xw = xw_pool.tile([128, chunk_t * 128], fp16, tag="xw")
                    src_rows = x_d[c * rpc : (c + 1) * rpc, :]
                    nc.gpsimd.dma_start(
                        out=xw[:].rearrange("p (t f) -> p t f", f=128),
                        in_=src_rows.rearrange("(t p) f -> p t f", p=128),
                    )
                    pw = pw_pool.tile([128, chunk_t * 256], fp16, tag="pw")
                    for t in range(chunk_t):
                        xt_ps = psA.tile([128, 128], fp16, tag="xt_ps")
                        nc.tensor.transpose(
                            xt_ps[:], xw[:, t * 128 : (t + 1) * 128], ident[:]
                        )
                        xt_sb = xt_pool.tile([128, 128], fp16, tag="xt_sb")
                        nc.scalar.copy(xt_sb[:], xt_ps[:])
                        pp = psB.tile([128, 256], f32, tag="pp")
                        nc.tensor.matmul(
                            pp[:], lhsT=xt_sb[:], rhs=wcatT[:], start=True, stop=False
                        )
                        nc.tensor.matmul(
                            pp[:], lhsT=onesh[:], rhs=b1cat[:], start=False, stop=True
                        )
                        if t % 2 == 0:
                            nc.vector.tensor_copy(pw[:, t * 256 : (t + 1) * 256], pp[:])
                        else:
                            nc.scalar.copy(pw[:, t * 256 : (t + 1) * 256], pp[:])
                    dst_rows = p_d[c * rpc : (c + 1) * rpc, :]
                    nc.sync.dma_start(
                        out=dst_rows.rearrange("(t p) f -> p t f", p=128),
                        in_=pw[:].rearrange("p (t f) -> p t f", f=256),
                    )

        for rep in range(n_reps_p2):
            # ---------------- phase 2 (raw): gather + tail -----------------
            w2b3 = w2rep.unsqueeze(1).to_broadcast([128, JPU, 128])
            base_i = rep * 16           # idx_sem: one bulk load per rep
            base_d = rep * NU
            ng_slot = NU // DEPTH
            nl_slot = NU // LBUF
            base_g = rep * 32 * ng_slot   # per gsems slot, 32 per use
            base_o = rep * 16 * nl_slot   # per olsems slot, 16 per use

            with nc.Block() as block:

                @block.gpsimd
                def _(gp, rep=rep, base_g=base_g, base_d=base_d, base_i=base_i):
                    gp.load_library(library_config.mlp)
                    gp.wait_ge(idx_sem, base_i + 16)
                    for u in range(NU):
                        gr = u // CPG
                        sb, db = gr // 4, gr % 4
                        k = u % DEPTH
                        if u >= DEPTH:
                            gp.wait_ge(dv, base_d + u - DEPTH + 1)
                        acol = (2 * u) * SPU
                        bcol = (2 * u + 1) * SPU
                        gp.dma_gather(
                            dstA[:, k * JPU : (k + 1) * JPU, :],
                            p_d[sb * BINR : (sb + 1) * BINR, 0:128],
                            idx_all[:, acol : acol + SPU],
                            QG, QG, 128, elem_step=256, single_packet=False,
                            queue_num=0,
                        ).then_inc(gsems[k], 16)
                        gp.dma_gather(
                            dstB[:, k * JPU : (k + 1) * JPU, :],
                            p_d[db * BINR : (db + 1) * BINR, 128:256],
                            idx_all[:, bcol : bcol + SPU],
                            QG, QG, 128, elem_step=256, single_packet=False,
                            queue_num=1,
                        ).then_inc(gsems[k], 16)
                    gp.wait_ge(dv, base_d + NU)

                @block.vector
                def _(vec, rep=rep, base_g=base_g, base_d=base_d, base_o=base_o):
                    if not tail:
                        for u in range(NU):
                            k = u % DEPTH
                            vec.wait_ge(gsems[k], base_g + 32 * (u // DEPTH + 1))
                            vec.drain().then_inc(dv, 1)
                        return
                    for u in range(NU):
                        k = u % DEPTH
                        lk = u % LBUF
                        vec.wait_ge(gsems[k], base_g + 32 * (u // DEPTH + 1))
                        sA = dstA[:, k * JPU : (k + 1) * JPU, :]
                        sB = dstB[:, k * JPU : (k + 1) * JPU, :]
                        vec.tensor_add(out=sA, in0=sA, in1=sB)
                        vec.drain()
                        vec.scalar_tensor_tensor(
                            out=sB, in0=sA, scalar=0.0, in1=w2b3,
                            op0=ALU.max, op1=ALU.mult,
                        )
                        vec.drain()
                        if u >= LBUF:
                            vec.wait_ge(olsems[lk], base_o + 16 * (u // LBUF))
                        with nc.allow_low_precision("fp16 partial reduce, 8 terms"):
                            vec.tensor_reduce(
                                out=red1[:].rearrange("p (j s) -> p j s", s=16),
                                in_=sB.rearrange("p j (s w) -> p j s w", w=8),
                                axis=mybir.AxisListType.X,
                                op=ALU.add,
                            )
                        vec.drain()
                        lslice = lg[:, lk * JPU : (lk + 1) * JPU]
                        vec.tensor_reduce(
                            out=lslice,
                            in_=red1[:].rearrange("p (j s) -> p j s", s=16),
                            axis=mybir.AxisListType.X,
                            op=ALU.add,
                        )
                        vec.drain()
                        vec.tensor_scalar_add(
                            out=lslice, in0=lslice, scalar1=b2bc[:, 0:1]
                        ).then_inc(dv, 1)
                    for lk in range(LBUF):
                        vec.wait_ge(olsems[lk], base_o + 16 * (NU // LBUF))

                @block.scalar
                def _(act, rep=rep, base_d=base_d, base_i=base_i, base_o=base_o):
                    act.dma_start(out=idx_all, in_=gidx_d).then_inc(idx_sem, 16)
                    if not tail:
                        act.wait_ge(dv, base_d + NU)
                        return
                    for u in range(NU):
                        lk = u % LBUF
                        act.wait_ge(dv, base_d + u + 1)
                        act.dma_start(
                            out=out_d[u * QG : (u + 1) * QG].rearrange(
                                "(p j) -> p j", p=128
                            ),
                            in_=lg[:, lk * JPU : (lk + 1) * JPU],
                        ).then_inc(olsems[lk], 16)
                    for lk in range(LBUF):
                        act.wait_ge(olsems[lk], base_o + 16 * (NU // LBUF))

    nc.compile()
    return nc


def make_in_maps(inputs, cfg=None, n_cores=N_CORES):
    """Shard, bin, and pad host inputs into per-core input maps.

    Returns (in_maps, pos_list) where pos_list[c] maps each original edge of
    core c to its device slot in the logits output.
    """
    cfg = cfg or FULL_CFG
    n_pad = cfg["n_pad"]
    BINR = cfg["bin_rows"]
    QG = cfg["qg"]
    CPG = cfg["calls_per_group"]
    NU = _units(cfg)
    SPU = QG // 16
    quota = CPG * QG

    x = np.asarray(inputs["block_outputs"], dtype=np.float32)
    n_nodes = x.shape[0]
    x_pad = np.zeros((n_pad, D), dtype=np.float32)
    x_pad[:n_nodes] = x

    src = np.asarray(inputs["src"]).astype(np.int64)
    dst = np.asarray(inputs["dst"]).astype(np.int64)
    e_total = src.shape[0]
    e_core = e_total // n_cores

    w1 = np.ascontiguousarray(np.asarray(inputs["W1"], dtype=np.float32))
    b1 = np.ascontiguousarray(np.asarray(inputs["b1"], dtype=np.float32))
    w2 = np.ascontiguousarray(np.asarray(inputs["W2"], dtype=np.float32))
    b2 = np.ascontiguousarray(np.asarray(inputs["b2"], dtype=np.float32))

    in_maps, pos_list = [], []
    for c in range(n_cores):
        s_c = src[c * e_core : (c + 1) * e_core]
        d_c = dst[c * e_core : (c + 1) * e_core]
        grp = (s_c // BINR) * 4 + (d_c // BINR)
        order = np.argsort(grp, kind="stable")
        counts = np.bincount(grp, minlength=16)
        assert counts.max() <= quota, f"group quota exceeded: {counts.max()}"

        # per-group padded local indices + device position of each edge
        gidx = np.zeros((128, NU * 2 * SPU), dtype=np.int16)
        pos = np.empty(e_core, dtype=np.int64)
        off = 0
        for gr in range(16):
            cnt = counts[gr]
            eids = order[off : off + cnt]
            off += cnt
            sl = np.zeros(quota, dtype=np.int16)
            dl = np.zeros(quota, dtype=np.int16)
            sl[:cnt] = (s_c[eids] - (gr // 4) * BINR).astype(np.int16)
            dl[:cnt] = (d_c[eids] - (gr % 4) * BINR).astype(np.int16)
            i = np.arange(cnt)
            u_loc = i // QG
            k = i % QG
            pos[eids] = (gr * CPG + u_loc) * QG + (k % 128) * (QG // 128) + k // 128
            # write wrapped idx tiles for each call of this group
            for cc in range(CPG):
                u = gr * CPG + cc
                a = sl[cc * QG : (cc + 1) * QG].reshape(QG // 16, 16).T
                b = dl[cc * QG : (cc + 1) * QG].reshape(QG // 16, 16).T
                gidx[:, (2 * u) * SPU : (2 * u + 1) * SPU] = np.tile(a, (8, 1))
                gidx[:, (2 * u + 1) * SPU : (2 * u + 2) * SPU] = np.tile(b, (8, 1))

        in_maps.append(
            {
                "x": x_pad, "w1": w1, "b1": b1, "w2": w2, "b2": b2,
                "gidx": np.ascontiguousarray(gidx),
            }
        )
        pos_list.append(pos)
    return in_maps, pos_list


_COMPILED = None


def kernel(**inputs):
    """Full-input entry point: shards across 8 NeuronCores, returns full output."""
    global _COMPILED
    from concourse.bass_utils import run_bass_kernel_spmd

    if _COMPILED is None:
        _COMPILED = build_bass(FULL_CFG)
    nc = _COMPILED

    in_maps, pos_list = make_in_maps(inputs, FULL_CFG)
    res = run_bass_kernel_spmd(nc, in_maps, core_ids=list(range(N_CORES))).results
    logits = np.concatenate(
        [res[c]["logits"][pos_list[c]] for c in range(N_CORES)]
    ).astype(np.float32)
    labels = np.ones_like(logits)
    return logits, labels



# revision 18
# speedup vs baseline: 2.0838x; 2.0838x over previous
"""Trainium2 Bass kernel: GNN edge decoder (nn_Decoder), SBUF-table design.

Computation (per edge e):
    emb  = concat(X[src[e]], X[dst[e]])          # [256]
    h    = relu(emb @ W1.T + b1)                 # [128]
    logit= h @ W2.T + b2                         # scalar
Outputs: (logits[E], labels[E]=ones)

Equivalent form used here: with A[n] = X[n]@W1a.T + b1 and B[n] = X[n]@W1b.T,
    logit_e = w2 . relu(A[src_e] + B[dst_e]) + b2.

Device strategy (8 cores = 4 src-quarters x 2 dst-halves):
  Core (qi, hj) owns edges with src in quarter qi (25088 padded nodes) and
  dst in half hj (50176 nodes = 2 sub-bins of 25088, int16-indexable).

Phase 1 (Tile): matmul X.T tiles (host-supplied fp16, pre-transposed,
  pre-sliced per core) against host-pretransposed W1a.T/W1b.T; PSUM -> SBUF.
  Both tables stay RESIDENT IN SBUF (A: 6.4MB, B: 12.8MB) - no DRAM round
  trip and no HBM traffic for the phase-2 gathers.

Phase 2 (raw): per 1024-edge unit,
  Pool: 2x SBUF-source dma_gather (transpose layout) -> gA,gB [128j, 1024e]
  DVE:  gA += gB
  ACT:  gB = relu(gA)
  PE:   8x matmul(lhsT=gB[:, c*128:(c+1)*128], rhs=w2col) -> psum[128e, col]
  DVE:  per 50ish-unit group: logits = psum + b2 -> SBUF
  SP:   streams idx chunks in, DMAs logits out (partition-major layout).
Host bins/pads edges per (core, dst sub-bin), builds wrapped int16 gather
indices, and un-permutes logits.
"""

import numpy as np

D = 128
N_NODES = 100000
E_TOTAL = 800000
N_CORES = 8

QTR = 25088            # nodes per src-quarter (= 196*128)
HALF = 2 * QTR         # nodes per dst-half
NPAD = 4 * QTR         # padded node count
RANKS = QTR // 128     # 196 sbuf-gather ranks per 25088-node table


def make_cfg(upg, qg=1024):
    return dict(
        upg=upg,                   # units (of qg edges) per dst sub-bin group
        qg=qg,                     # edges per dma_gather call
        depth=3 if qg <= 1024 else 2,   # gather units in flight
        cu=16384 // qg,            # units per idx stream chunk
    )


FULL_CFG = make_cfg(25, qg=2048)    # default; kernel() re-derives from data


def build_bass(cfg, n_reps=1, n_reps_p1=None, n_reps_p2=None, tail=True,
               debug_dump=False, evac_late=False):
    from contextlib import ExitStack

    import concourse.bacc as bacc
    import concourse.tile as tile
    from concourse import bass, library_config, mybir

    UPG = cfg["upg"]
    QG = cfg["qg"]
    DEPTH = cfg["depth"]
    CU = cfg["cu"]
    NU = 2 * UPG
    SPU = QG // 16
    JC = QG // 128
    NSLOTS = NU * QG
    NCH = -(-NU // CU)
    assert UPG * JC <= 512, "psum bank overflow"

    fp16 = mybir.dt.float16
    f32 = mybir.dt.float32
    i16 = mybir.dt.int16

    if n_reps_p1 is None:
        n_reps_p1 = n_reps
    if n_reps_p2 is None:
        n_reps_p2 = n_reps

    nc = bacc.Bacc(
        "TRN2", target_bir_lowering=False, debug=False, num_devices=N_CORES,
        num_swdge_queues=2, dynamic_dma_scratch_size=32768,
    )

    x_d = nc.dram_tensor("x", [128, QTR + HALF], fp16, kind="ExternalInput").ap()
    w1at_d = nc.dram_tensor("w1at", [128, 128], fp16, kind="ExternalInput").ap()
    w1bt_d = nc.dram_tensor("w1bt", [128, 128], fp16, kind="ExternalInput").ap()
    b1_d = nc.dram_tensor("b1r", [1, 128], fp16, kind="ExternalInput").ap()
    w2c_d = nc.dram_tensor("w2c", [128, 1], fp16, kind="ExternalInput").ap()
    b2c_d = nc.dram_tensor("b2c", [128, 1], f32, kind="ExternalInput").ap()
    gidx_d = nc.dram_tensor("gidx", [128, NU * 2 * SPU], i16, kind="ExternalInput").ap()
    out_d = nc.dram_tensor("logits", [NSLOTS], f32, kind="ExternalOutput").ap()

    # persistent SBUF
    atab_t = nc.alloc_sbuf_tensor("atab", [128, QTR], fp16)
    btab_t = nc.alloc_sbuf_tensor("btab", [128, HALF], fp16)
    bufA_t = nc.alloc_sbuf_tensor("bufA", [128, DEPTH, QG], fp16)
    bufB_t = nc.alloc_sbuf_tensor("bufB", [128, DEPTH, QG], fp16)
    idxb_t = nc.alloc_sbuf_tensor("idxb", [128, 2, CU * 2 * SPU], i16)
    lg_t = nc.alloc_sbuf_tensor("lg", [128, 2, UPG * JC], f32)
    w2c_t = nc.alloc_sbuf_tensor("w2cs", [128, 1], fp16)
    b2c_t = nc.alloc_sbuf_tensor("b2cs", [128, 1], f32)
    z1_t = nc.alloc_sbuf_tensor("z1s", [128, 1], fp16)
    dbg_t = nc.alloc_sbuf_tensor("dbgs", [128, NU, 2], fp16)
    atab = atab_t.ap()
    btab = btab_t.ap()
    bufA = bufA_t.ap()
    bufB = bufB_t.ap()
    idxb = idxb_t.ap()
    lg = lg_t.ap()
    w2c = w2c_t.ap()
    b2c = b2c_t.ap()
    z1 = z1_t.ap()
    dbg = dbg_t.ap()

    ps = [
        nc.alloc_psum_tensor("ps0", [128, UPG * JC], f32).ap(),
        nc.alloc_psum_tensor("ps1", [128, UPG * JC], f32).ap(),
    ]

    # running semaphore totals (persist across reps)
    g_tot = [0] * DEPTH
    a_tot = [0] * DEPTH
    r_tot = [0] * DEPTH
    dv_tot = [0]
    p_tot = [0]
    ev_tot = [0]
    o_tot = [0]
    i_tot = [0]

    with ExitStack() as top:
        idx_sem = top.enter_context(nc.semaphore("idx_sem"))
        dv = top.enter_context(nc.semaphore("dv"))
        evsem = top.enter_context(nc.semaphore("evsem"))
        osem = top.enter_context(nc.semaphore("osem"))
        psem = top.enter_context(nc.semaphore("psem"))
        gsems = [top.enter_context(nc.semaphore(f"gs{k}")) for k in range(DEPTH)]
        asems = [top.enter_context(nc.semaphore(f"as{k}")) for k in range(DEPTH)]
        rsems = [top.enter_context(nc.semaphore(f"rs{k}")) for k in range(DEPTH)]

        for rep in range(n_reps_p1):
            # ---------------- phase 1 (Tile): build tables in SBUF ---------
            with ExitStack() as ctx:
                tc = ctx.enter_context(tile.TileContext(nc))
                const = ctx.enter_context(tc.tile_pool(name="const", bufs=1))

                w1aT = const.tile([128, 128], fp16)
                nc.sync.dma_start(out=w1aT[:], in_=w1at_d)
                w1bT = const.tile([128, 128], fp16)
                nc.sync.dma_start(out=w1bT[:], in_=w1bt_d)
                b1row = const.tile([1, 128], fp16)
                nc.sync.dma_start(out=b1row[:], in_=b1_d)
                ones1 = const.tile([1, 128], fp16)
                nc.vector.memset(ones1[:], 1.0)
                nc.sync.dma_start(out=w2c, in_=w2c_d)
                nc.sync.dma_start(out=b2c, in_=b2c_d)
                nc.vector.memset(z1, 0.0)

                xw_pool = ctx.enter_context(tc.tile_pool(name="xw", bufs=2))
                psA = ctx.enter_context(
                    tc.tile_pool(name="psA", bufs=3, space="PSUM")
                )

                CT = 16  # node tiles per load chunk

                def table_pass(n_tiles, col0, rhs, bias, dst):
                    # dst: function tile-index -> destination AP slice
                    done = 0
                    ev = 0
                    while done < n_tiles:
                        nt = min(CT, n_tiles - done)
                        xw = xw_pool.tile([128, nt * 128], fp16, tag="xw")
                        nc.sync.dma_start(
                            out=xw[:],
                            in_=x_d[:, col0 + done * 128 : col0 + (done + nt) * 128],
                        )
                        j = 0
                        while j < nt:
                            nq = min(4, nt - j)
                            pp = psA.tile([128, nq * 128], f32, tag="pp")
                            for q in range(nq):
                                nc.tensor.matmul(
                                    pp[:, q * 128 : (q + 1) * 128],
                                    lhsT=xw[:, (j + q) * 128 : (j + q + 1) * 128],
                                    rhs=rhs[:],
                                    start=True,
                                    stop=(bias is None),
                                )
                                if bias is not None:
                                    nc.tensor.matmul(
                                        pp[:, q * 128 : (q + 1) * 128],
                                        lhsT=ones1[:],
                                        rhs=bias[:],
                                        start=False,
                                        stop=True,
                                    )
                            dslice = dst(done + j, nq)
                            if ev % 2 == 0:
                                nc.vector.tensor_copy(out=dslice, in_=pp[:])
                            else:
                                nc.scalar.copy(dslice, pp[:])
                            ev += 1
                            j += nq
                        done += nt

                table_pass(
                    RANKS, 0, w1aT, b1row,
                    lambda t, n: atab[:, t * 128 : (t + n) * 128],
                )
                table_pass(
                    2 * RANKS, QTR, w1bT, None,
                    lambda t, n: btab[:, t * 128 : (t + n) * 128],
                )

        for rep in range(n_reps_p2):
            # ---------------- phase 2 (raw): gather + tail -----------------
            dv_base = dv_tot[0]
            i_base = i_tot[0]
            ev_base = ev_tot[0]
            o_base = o_tot[0]
            g_base = list(g_tot)
            a_base = list(a_tot)
            r_base = list(r_tot)

            # per-unit schedule values
            u_chunk = [u // CU for u in range(NU)]
            _cnt = list(g_tot)
            lvl = []
            for v in range(NU):
                _cnt[v % DEPTH] += 32
                lvl.append(_cnt[v % DEPTH])
            ev_after = [min((g + 1) * UPG + DEPTH + 1, NU - 1) for g in range(2)]
            if evac_late:
                ev_after = [NU - 1, NU - 1]

            with nc.Block() as block:

                @block.gpsimd
                def _(gp, dv_base=dv_base, i_base=i_base):
                    gp.load_library(library_config.mlp)
                    last_c = -1
                    for u in range(NU):
                        k = u % DEPTH
                        g = u // UPG
                        c = u_chunk[u]
                        if c != last_c:
                            gp.wait_ge(idx_sem, i_base + 16 * (c + 1))
                            last_c = c
                        if u >= DEPTH:
                            gp.wait_ge(dv, dv_base + u - DEPTH + 1)
                        lu = u - c * CU
                        acol = (2 * lu) * SPU
                        bcol = (2 * lu + 1) * SPU
                        gp.dma_gather(
                            bufA[:, k : k + 1, :],
                            atab,
                            idxb[:, c % 2, acol : acol + SPU],
                            QG, QG, 128,
                            transpose=True, single_packet=False, queue_num=0,
                            sbuf_tokens_per_rank=128,
                            sbuf_free_dim_per_rank=256,
                            prepare_only=True, sem=gsems[k],
                        ).then_inc(psem, 1)
                        p_tot[0] += 1
                        gp.wait_ge(psem, p_tot[0])
                        gp.trigger_dma(count=1, queue_num=0)
                        g_tot[k] += 16
                        gp.dma_gather(
                            bufB[:, k : k + 1, :],
                            btab[:, g * QTR : (g + 1) * QTR],
                            idxb[:, c % 2, bcol : bcol + SPU],
                            QG, QG, 128,
                            transpose=True, single_packet=False, queue_num=1,
                            sbuf_tokens_per_rank=128,
                            sbuf_free_dim_per_rank=256,
                            prepare_only=True, sem=gsems[k],
                        ).then_inc(psem, 1)
                        p_tot[0] += 1
                        gp.wait_ge(psem, p_tot[0])
                        gp.trigger_dma(count=1, queue_num=1)
                        g_tot[k] += 16
                    gp.wait_ge(dv, dv_base + NU)

                @block.vector
                def _(vec, dv_base=dv_base, g_base=g_base, rep=rep):
                    for u in range(NU):
                        k = u % DEPTH
                        vec.wait_ge(gsems[k], lvl[u])
                        sA = bufA[:, k : k + 1, :]
                        sB = bufB[:, k : k + 1, :]
                        if tail:
                            vec.tensor_add(out=sA, in0=sA, in1=sB).then_inc(
                                asems[k], 1
                            )
                            a_tot[k] += 1
                            if debug_dump:
                                vec.tensor_copy(out=dbg[:, u, 0:1],
                                                in_=bufA[:, k, 0:1])
                        else:
                            vec.drain().then_inc(dv, 1)
                            dv_tot[0] += 1
                        if tail:
                            for g in range(2):
                                if u == ev_after[g]:
                                    vec.wait_ge(dv, dv_base + (g + 1) * UPG)
                                    vec.tensor_scalar_add(
                                        out=lg[:, g, :], in0=ps[g], scalar1=b2c
                                    ).then_inc(evsem, 1)
                                    ev_tot[0] += 1

                if tail:

                    @block.scalar
                    def _(act, a_base=a_base):
                        aseen = list(a_base)
                        for u in range(NU):
                            k = u % DEPTH
                            aseen[k] += 1
                            act.wait_ge(asems[k], aseen[k])
                            mm = act.activation(
                                out=bufB[:, k : k + 1, :],
                                in_=bufA[:, k : k + 1, :],
                                func=mybir.ActivationFunctionType.Relu,
                                bias=z1,
                            )
                            if debug_dump:
                                act.copy(dbg[:, u, 1:2], bufB[:, k, 0:1])
                                act.drain().then_inc(rsems[k], 1)
                            else:
                                mm.then_inc(rsems[k], 1)
                            r_tot[k] += 1

                    @block.tensor
                    def _(te, r_base=r_base):
                        rseen = list(r_base)
                        for u in range(NU):
                            k = u % DEPTH
                            g = u // UPG
                            ul = u - g * UPG
                            rseen[k] += 1
                            te.wait_ge(rsems[k], rseen[k])
                            for c in range(JC):
                                mm = te.matmul(
                                    ps[g][:, ul * JC + c : ul * JC + c + 1],
                                    lhsT=bufB[:, k, c * 128 : (c + 1) * 128],
                                    rhs=w2c,
                                    start=True,
                                    stop=True,
                                )
                            mm.then_inc(dv, 1)
                            dv_tot[0] += 1

                @block.sync
                def _(sp, dv_base=dv_base, i_base=i_base, ev_base=ev_base,
                      o_base=o_base):
                    for c in range(NCH):
                        if c >= 2:
                            sp.wait_ge(dv, dv_base + (c - 1) * CU)
                        u0 = c * CU
                        u1 = min(NU, (c + 1) * CU)
                        ncols = (u1 - u0) * 2 * SPU
                        sp.dma_start(
                            out=idxb[:, c % 2, 0:ncols],
                            in_=gidx_d[:, u0 * 2 * SPU : u1 * 2 * SPU],
                        ).then_inc(idx_sem, 16)
                        i_tot[0] += 16
                    if tail:
                        odview = out_d.rearrange("(p x) -> p x", p=128)
                        for g in range(2):
                            sp.wait_ge(evsem, ev_base + g + 1)
                            sp.dma_start(
                                out=odview[:, g * UPG * JC : (g + 1) * UPG * JC],
                                in_=lg[:, g, :],
                            ).then_inc(osem, 16)
                            o_tot[0] += 16
                        sp.wait_ge(osem, o_base + 32)
                    else:
                        sp.wait_ge(dv, dv_base + NU)

        if debug_dump:
            dsem = top.enter_context(nc.semaphore("dsem"))
            dumps = [
                ("d_atab", atab, [128, QTR], fp16),
                ("d_btab", btab, [128, HALF], fp16),
                ("d_bufA", bufA, [128, DEPTH, QG], fp16),
                ("d_bufB", bufB, [128, DEPTH, QG], fp16),
                ("d_idxb", idxb, [128, 2, CU * 2 * SPU], i16),
                ("d_lg", lg, [128, 2, UPG * JC], f32),
                ("d_dbg", dbg, [128, NU, 2], fp16),
            ]
            with nc.Block() as blk:

                @blk.sync
                def _(sp):
                    cnt = 0
                    for name, src_ap, shape, dt in dumps:
                        dd = nc.dram_tensor(
                            name, shape, dt, kind="ExternalOutput"
                        ).ap()
                        sp.dma_start(out=dd, in_=src_ap).then_inc(dsem, 16)
                        cnt += 16
                    sp.wait_ge(dsem, cnt)

    nc.compile()
    return nc


def prepare(inputs):
    """Shard/bin/pad host inputs. Returns (in_maps, recover, cfg)."""
    x = np.asarray(inputs["block_outputs"], dtype=np.float32)
    src = np.asarray(inputs["src"]).astype(np.int64)
    dst = np.asarray(inputs["dst"]).astype(np.int64)
    w1 = np.asarray(inputs["W1"], dtype=np.float32)
    b1 = np.asarray(inputs["b1"], dtype=np.float32)
    w2 = np.asarray(inputs["W2"], dtype=np.float32)
    b2 = np.asarray(inputs["b2"], dtype=np.float32)

    xt = np.zeros((128, NPAD), dtype=np.float16)
    xt[:, : x.shape[0]] = x.T.astype(np.float16)
    w1at = np.ascontiguousarray(w1[:, :128].T.astype(np.float16))
    w1bt = np.ascontiguousarray(w1[:, 128:].T.astype(np.float16))
    b1r = np.ascontiguousarray(b1.reshape(1, 128).astype(np.float16))
    w2c = np.ascontiguousarray(w2.reshape(128, 1).astype(np.float16))
    b2c = np.full((128, 1), float(b2.reshape(-1)[0]), dtype=np.float32)

    qi_all = src // QTR
    hj_all = dst // HALF
    core_all = qi_all * 2 + hj_all

    per_core = []
    max_cnt = 0
    for c in range(N_CORES):
        qi, hj = c // 2, c % 2
        eids = np.nonzero(core_all == c)[0]
        s_l = (src[eids] - qi * QTR).astype(np.int64)
        d_half = dst[eids] - hj * HALF
        g = d_half // QTR
        order = np.argsort(g, kind="stable")
        eids = eids[order]
        s_l = s_l[order]
        d_l = (d_half[order] - g[order] * QTR).astype(np.int64)
        cnt = np.bincount(g, minlength=2)
        max_cnt = max(max_cnt, int(cnt.max()))
        per_core.append((eids, s_l, d_l, cnt))

    import os
    qg = int(os.environ.get("QG", "2048"))
    cfg = make_cfg(-(-max_cnt // qg), qg=qg)
    UPG, QG = cfg["upg"], cfg["qg"]
    NU = 2 * UPG
    SPU = QG // 16
    JC = QG // 128
    quota = UPG * QG

    in_maps, recover = [], []
    for c in range(N_CORES):
        qi, hj = c // 2, c % 2
        eids, s_l, d_l, cnt = per_core[c]
        gidx = np.zeros((128, NU * 2 * SPU), dtype=np.int16)
        pos = np.empty(eids.shape[0], dtype=np.int64)
        off = 0
        for g in range(2):
            n = int(cnt[g])
            sl = np.zeros(quota, dtype=np.int16)
            dl = np.zeros(quota, dtype=np.int16)
            sl[:n] = s_l[off : off + n].astype(np.int16)
            dl[:n] = d_l[off : off + n].astype(np.int16)
            i = np.arange(n)
            k = i % QG
            u_loc = i // QG
            pos[off : off + n] = (
                (k % 128) * (NU * JC) + (g * UPG + u_loc) * JC + (k // 128)
            )
            for u_loc2 in range(UPG):
                u = g * UPG + u_loc2
                a = sl[u_loc2 * QG : (u_loc2 + 1) * QG].reshape(SPU, 16).T
                bb = dl[u_loc2 * QG : (u_loc2 + 1) * QG].reshape(SPU, 16).T
                gidx[:, (2 * u) * SPU : (2 * u + 1) * SPU] = np.tile(a, (8, 1))
                gidx[:, (2 * u + 1) * SPU : (2 * u + 2) * SPU] = np.tile(
                    bb, (8, 1)
                )
            off += n
        x_local = np.ascontiguousarray(
            np.concatenate(
                [xt[:, qi * QTR : (qi + 1) * QTR],
                 xt[:, hj * HALF : (hj + 1) * HALF]],
                axis=1,
            )
        )
        in_maps.append(
            {
                "x": x_local, "w1at": w1at, "w1bt": w1bt, "b1r": b1r,
                "w2c": w2c, "b2c": b2c, "gidx": np.ascontiguousarray(gidx),
            }
        )
        recover.append((eids, pos))
    return in_maps, recover, cfg


_COMPILED = {}


def kernel(**inputs):
    """Full-input entry point: shards across 8 NeuronCores, returns full output."""
    from concourse.bass_utils import run_bass_kernel_spmd

    in_maps, recover, cfg = prepare(inputs)
    key = cfg["upg"]
    if key not in _COMPILED:
        _COMPILED[key] = build_bass(cfg)
    nc = _COMPILED[key]

    res = run_bass_kernel_spmd(nc, in_maps, core_ids=list(range(N_CORES))).results
    logits = np.empty(E_TOTAL, dtype=np.float32)
    for c in range(N_CORES):
        eids, pos = recover[c]
        logits[eids] = res[c]["logits"][pos]
    labels = np.ones_like(logits)
    return logits, labels
